# revision 13
# baseline (speedup 1.0000x reference)
"""MoE (noisy top-k gating, Shazeer-style) Trainium2 kernel.

Strategy (expert parallelism, per the sharding hint):
  - Gating (x@w_gate, noisy logits, top-4, softmax) runs on host in fp32
    numpy: it is 0.02% of the FLOPs and produces the routing needed to
    shard ("all-to-all dispatch" done host-side since I/O is full anyway).
  - The 16 experts' weights are sharded 2-per-core across 8 NeuronCores.
  - Fast path (b1 == 0, the case produced by setup_inputs): the softmax
    gate g>0 is folded into the dispatched tokens on the host
    (relu(g*x@W1) = g*relu(x@W1) since b1==0), so the device computes
      hT[H,C] = relu(W1^T @ xg^T)          (C = padded token capacity)
      yT[D,C] = W2^T @ hT                  (already gate-scaled)
    with C as the matmul *moving* dim in both layers. That allows exact
    token capacities (multiple of 8, not 128): C0=slot0 max load,
    C1=slot1 max load, cutting padded FLOPs vs the 128-aligned layout.
    Custom tile loops stream W1/W2 from HBM; xT and hT stay resident in
    SBUF; yT is written back in bf16.
  - General path (b1 != 0): original composable-matmul structure with
    on-device bias+gate.
  - Host scatters per-expert outputs back and adds the gates @ b2 term.

Shapes are hardcoded for B=4096, D=1024, H=4096, E=16, TOP_K=4.
"""

import numpy as np
import ml_dtypes

import concourse.bass as bass
import concourse.mybir as mybir
import concourse.tile as tile
from concourse import bacc
from concourse.bass import ds, ts
from concourse.bass_utils import run_bass_kernel_spmd
from concourse.kernels.tile_matmul import (
    ShapeInfo,
    composable_matmul_tile_kernel,
    dma_from_dram_kxm,
    dma_from_dram_kxn,
    dma_to_dram_mxn,
    k_pool_min_bufs_for_dim,
)

B, D, H, E, TOP_K, NCORES = 4096, 1024, 4096, 16, 4, 8
EPC = E // NCORES  # experts per core
KO1 = D // 128  # k-subtiles for layer 1 (8)
KO2 = H // 128  # k-subtiles for layer 2 (32)
BF16 = mybir.dt.bfloat16
F32 = mybir.dt.float32
AF = mybir.ActivationFunctionType

# Results of the last device run (exec_time_ns etc.), for test harnesses.
LAST_RESULTS = None


def _gating(x, noise, w_gate, w_noise, b_noise):
    """Mirror of the reference gating in fp32 numpy.

    Verified on the actual inputs: the top-4 sets match jax-CPU bitwise
    selection (min 4th/5th logit gap 5.7e-5 vs <2e-6 numeric diff).
    """
    clean = x @ w_gate
    stddev = np.logaddexp(0.0, x @ w_noise + b_noise).astype(np.float32)
    noisy = clean + noise * stddev
    order = np.argsort(-noisy, axis=1, kind="stable")[:, :TOP_K]
    top_vals = np.take_along_axis(noisy, order, axis=1)
    ex = np.exp(top_vals - top_vals.max(axis=1, keepdims=True))
    top_gates = (ex / ex.sum(axis=1, keepdims=True)).astype(np.float32)
    return order, top_gates


def _chunks(C, first=None, last=None):
    """Split C into pieces in [128, 512], multiples of 8.

    `first`/`last` force a small piece at that end (head/tail latency).
    """
    pieces = []
    rem = C
    if first:
        pieces.append(first)
        rem -= first
    tail = [last] if last else []
    if last:
        rem -= last
    n = max(1, (rem + 511) // 512)
    base = rem // n // 8 * 8
    mid = [base] * n
    for i in range((rem - base * n) // 8):
        mid[i % n] += 8
    assert all(128 <= p <= 512 for p in pieces + mid + tail), (C, pieces, mid, tail)
    out = pieces + mid + tail
    assert sum(out) == C
    return out


def _build_program_fast(Cs):
    """Per-core SPMD program, fast path (no b1, gate folded into x).

    Per expert slot j (capacity C=Cs[j]):
      inputs  w1_j [D,H] bf16, w2_j [H,D] bf16, xt_j [D,C] bf16 (gated)
      output  yt_j [D,C] bf16
    xT and hT are SBUF-resident; W1/W2 stream from HBM per 128-row
    m-block; C is the moving dim of every matmul, chunked into pieces
    <=512 (PSUM bank width). hT buffer is shared by both experts.
    """
    from contextlib import ExitStack

    nc = bacc.Bacc(None, target_bir_lowering=False)
    in_names = {}
    out_names = {}
    with ExitStack() as ctx:
        tc = ctx.enter_context(tile.TileContext(nc))
        dram = ctx.enter_context(tc.tile_pool(name="dram", bufs=1, space="DRAM"))
        const = ctx.enter_context(tc.tile_pool(name="const", bufs=1))
        w1p = ctx.enter_context(tc.tile_pool(name="w1p", bufs=8))
        w2p = ctx.enter_context(tc.tile_pool(name="w2p", bufs=4))
        yp = ctx.enter_context(tc.tile_pool(name="yp", bufs=4))
        psum = ctx.enter_context(tc.tile_pool(name="psum", bufs=4, space="PSUM"))

        ins = {}
        outs = {}
        for j in range(EPC):
            C = Cs[j]
            ins[f"w1_{j}"] = dram.tile([D, H], BF16, kind="ExternalInput", name=f"w1_{j}")
            ins[f"w2_{j}"] = dram.tile([H, D], BF16, kind="ExternalInput", name=f"w2_{j}")
            ins[f"xt_{j}"] = dram.tile([D, C], BF16, kind="ExternalInput", name=f"xt_{j}")
            outs[f"yt_{j}"] = dram.tile([D, C], BF16, kind="ExternalOutput", name=f"yt_{j}")
        for key, ap in ins.items():
            in_names[key] = ap.tensor.name
        for key, ap in outs.items():
            out_names[key] = ap.tensor.name

        xt_sbs = [const.tile([128, KO1, Cs[j]], BF16, name=f"xts{j}") for j in range(EPC)]
        hT_sb = const.tile([128, KO2, Cs[0]], BF16, name="hts")

        def _load_xt_head(j, chunks):
            # DMA issue costs ~0.6us of sequencer time per dma_start, so the
            # critical first-expert load alternates issues between the two
            # otherwise-idle engines (scalar=Activation HWDGE, gpsimd) while
            # sync feeds W1. Chunk 0 lands first so the PE can start.
            xt3d = ins[f"xt_{j}"].rearrange("(ko p) c -> p ko c", p=128)
            engines = [nc.scalar, nc.gpsimd]
            n = 0
            starts = [sum(chunks[:i]) for i in range(len(chunks))]
            # Single-ko pieces: full chunk width keeps DRAM segments ~1KB
            # (2-ko pieces halve per-queue DMA bandwidth). Chunk 0 is issued
            # LAST, on three engines: the PE's first matmul waits on it, so
            # the PE starts only when ALL of xt is resident and then runs
            # gap-free (stalling mid-stream costs p-state re-ramp).
            for ci in list(range(1, len(chunks))) + [0]:
                cw, c0 = chunks[ci], starts[ci]
                cs = ds(c0, cw)
                for ko in range(KO1):
                    eng = engines[n % 2]
                    n += 1
                    eng.dma_start(xt_sbs[j][:, ko, cs], xt3d[:, ko, cs])

        def _load_xt(j, chunks):
            # non-critical prefetch: few pieces, on the idle gpsimd engine
            xt3d = ins[f"xt_{j}"].rearrange("(ko p) c -> p ko c", p=128)
            c0 = 0
            for cw in chunks:
                for kh in range(0, KO1, 4):
                    ks = ds(kh, 4)
                    nc.gpsimd.dma_start(
                        xt_sbs[j][:, ks, ds(c0, cw)], xt3d[:, ks, ds(c0, cw)]
                    )
                c0 += cw

        ch_l1 = [_chunks(Cs[0], first=128), _chunks(Cs[1])]
        ch_l2 = [_chunks(Cs[0]), _chunks(Cs[1])]

        _load_xt_head(0, ch_l1[0])

        for j in range(EPC):
            C = Cs[j]

            # ---- layer 1: hT[:, :, :C] = relu(W1^T @ xgT) ----
            w1d = ins[f"w1_{j}"].rearrange("(ko p) h -> p ko h", p=128)
            for m in range(H // 128):
                w1t = w1p.tile([128, KO1, 128], BF16, name="w1t")
                # finer pieces for the very first block: it gates the PE start
                nk = 4 if (j == 0 and m == 0) else 2
                for kh in range(nk):
                    ks = ds(kh * (KO1 // nk), KO1 // nk)
                    nc.sync.dma_start(w1t[:, ks, :], w1d[:, ks, ds(m * 128, 128)])
                c0 = 0
                for cw in ch_l1[j]:
                    ps = psum.tile([128, 512], F32, name="ps")[:, :cw]
                    for ko in range(KO1):
                        nc.tensor.matmul(
                            ps,
                            w1t[:, ko, :],
                            xt_sbs[j][:, ko, ds(c0, cw)],
                            start=(ko == 0),
                            stop=(ko == KO1 - 1),
                        )
                    nc.scalar.activation(hT_sb[:, m, ds(c0, cw)], ps, AF.Relu)
                    c0 += cw

            # prefetch next expert's tokens while this one computes
            if j + 1 < EPC:
                _load_xt(j + 1, ch_l1[j + 1])

            # ---- layer 2: yT = W2^T @ hT (gate already folded in) ----
            w2d = ins[f"w2_{j}"].rearrange("(ko p) d -> p ko d", p=128)
            ytd = outs[f"yt_{j}"]
            for m in range(D // 128):
                w2t = w2p.tile([128, KO2, 128], BF16, name="w2t")
                for kq in range(4):
                    ks = ds(kq * (KO2 // 4), KO2 // 4)
                    nc.sync.dma_start(w2t[:, ks, :], w2d[:, ks, ds(m * 128, 128)])
                c0 = 0
                for cw in ch_l2[j]:
                    ps = psum.tile([128, 512], F32, name="ps")[:, :cw]
                    for ko in range(KO2):
                        nc.tensor.matmul(
                            ps,
                            w2t[:, ko, :],
                            hT_sb[:, ko, ds(c0, cw)],
                            start=(ko == 0),
                            stop=(ko == KO2 - 1),
                        )
                    yt = yp.tile([128, 512], BF16, name="yt")[:, :cw]
                    nc.scalar.activation(yt, ps, AF.Copy)
                    last = (
                        j == EPC - 1
                        and m == D // 128 - 1
                        and c0 + cw == Cs[j]
                    )
                    if last:
                        # final drain of the program: spread the writeback
                        # over three issue engines so the tail is short
                        step = (cw // 3 + 7) // 8 * 8
                        for i, eng in enumerate([nc.sync, nc.scalar, nc.gpsimd]):
                            o = i * step
                            if o >= cw:
                                break
                            w = min(step, cw - o)
                            eng.dma_start(
                                ytd[ds(m * 128, 128), ds(c0 + o, w)], yt[:, ds(o, w)]
                            )
                    else:
                        for o in range(0, cw, 256):
                            w = min(256, cw - o)
                            nc.sync.dma_start(
                                ytd[ds(m * 128, 128), ds(c0 + o, w)], yt[:, ds(o, w)]
                            )
                    c0 += cw
    nc.compile()
    return nc, in_names, out_names


# ---------------------------------------------------------------------------
# General path (b1 != 0): original composable-matmul structure.
# ---------------------------------------------------------------------------


def _relu_bias_reducer(b1_sb):
    def f(nc, psum, sbuf, md):
        hb = md.m_tile_idx * md.m_subtiles + md.m_subtile_idx
        nc.scalar.activation(
            sbuf[:, 0, :], psum[:], AF.Relu, bias=b1_sb[:, hb : hb + 1]
        )

    return f


def _gate_scale_reducer(g_sb):
    def f(nc, psum, sbuf, md):
        mb = md.m_tile_idx * md.m_subtiles + md.m_subtile_idx
        nc.scalar.activation(
            sbuf[:, 0, :], psum[:], AF.Copy, scale=g_sb[:, mb : mb + 1]
        )

    return f


def _noop_consumer(nc, t, md):
    pass


def _c_tile(C):
    for t in (512, 384, 256, 128):
        if C % t == 0:
            return t
    raise AssertionError(C)


def _build_program_general(Cs):
    """SPMD per-core program: 2 experts, each a 2-layer FFN (with bias)."""
    from contextlib import ExitStack

    nc = bacc.Bacc(None, target_bir_lowering=False)
    in_names = {}
    out_names = {}
    with ExitStack() as ctx:
        tc = ctx.enter_context(tile.TileContext(nc))
        dram = ctx.enter_context(tc.tile_pool(name="dram", bufs=1, space="DRAM"))
        const = ctx.enter_context(tc.tile_pool(name="const", bufs=1))

        ins = {}
        outs = {}
        for j in range(EPC):
            C = Cs[j]
            ins[f"w1_{j}"] = dram.tile([D, H], BF16, kind="ExternalInput", name=f"w1_{j}")
            ins[f"w2_{j}"] = dram.tile([H, D], BF16, kind="ExternalInput", name=f"w2_{j}")
            ins[f"xt_{j}"] = dram.tile([D, C], BF16, kind="ExternalInput", name=f"xt_{j}")
            ins[f"b1_{j}"] = dram.tile([128, H // 128], F32, kind="ExternalInput", name=f"b1_{j}")
            ins[f"g_{j}"] = dram.tile([128, C // 128], F32, kind="ExternalInput", name=f"g_{j}")
            outs[f"y_{j}"] = dram.tile([C, D], F32, kind="ExternalOutput", name=f"y_{j}")

        for key, ap in ins.items():
            in_names[key] = ap.tensor.name
        for key, ap in outs.items():
            out_names[key] = ap.tensor.name

        xt_sbs = []
        w1_pools = []
        for j in range(EPC):
            C = Cs[j]
            xt_sbs.append(const.tile([128, D // 128, C], BF16, name=f"xts{j}"))
            w1_pools.append(
                ctx.enter_context(
                    tc.tile_pool(name=f"w1p{j}", bufs=k_pool_min_bufs_for_dim(D) + 1)
                )
            )

        def _load_xt(j):
            C = Cs[j]
            CT = _c_tile(C)
            xt3d = ins[f"xt_{j}"].rearrange("(ko p) c -> p ko c", p=128)
            for cb in range(C // CT):
                cs = ds(cb * CT, CT)
                for kt in range(2):
                    ks = ds(kt * (D // 256), D // 256)
                    nc.sync.dma_start(xt_sbs[j][:, ks, cs], xt3d[:, ks, cs])

        _load_xt(0)

        for j in range(EPC):
            C = Cs[j]
            CT = _c_tile(C)  # exact N tile for layer 1
            b1_sb = const.tile([128, H // 128], F32, name=f"b1sb{j}")
            nc.sync.dma_start(b1_sb[:], ins[f"b1_{j}"][:])
            g_sb = const.tile([128, C // 128], F32, name=f"gsb{j}")
            nc.sync.dma_start(g_sb[:], ins[f"g_{j}"][:])

            with ExitStack() as ectx:
                pers = ectx.enter_context(tc.tile_pool(name=f"pers{j}", bufs=1))
                xt_sb = xt_sbs[j]
                hT_sb = pers.tile([128, H // 128, C], BF16, name=f"hts{j}")

                def xt_producer(nc_, md, xt_sb=xt_sb):
                    return xt_sb[
                        :,
                        ts(md.k_tile_idx, md.k_subtiles),
                        ds(md.n_tile_idx * md.n_tile, md.n_tile),
                    ]

                def hT_out_producer(nc_, md, hT_sb=hT_sb):
                    return hT_sb[
                        :,
                        ds(md.m_tile_idx * md.m_subtiles, md.m_subtiles),
                        ds(md.n_tile_idx * md.n_tile, md.n_tile),
                    ]

                def hT_kxm_producer(nc_, md, hT_sb=hT_sb):
                    return hT_sb[
                        :,
                        ts(md.k_tile_idx, md.k_subtiles),
                        ds(md.m_tile_idx * md.m_tile, md.m_tile),
                    ]

                # layer 1: hT[H, C] = relu(W1[D,H].T @ xT[D,C] + b1)
                tc.swap_default_side()
                with ExitStack() as mctx:
                    w1_producer, w1_shape = dma_from_dram_kxm(
                        w1_pools[j], ins[f"w1_{j}"][:]
                    )
                    composable_matmul_tile_kernel(
                        tc=tc,
                        kxm_shape=w1_shape,
                        kxn_shape=ShapeInfo(pdims=((128, D // 128),), fdims=(C,)),
                        output_type=BF16,
                        kxm_producer=w1_producer,
                        kxn_producer=xt_producer,
                        mxn_subtile_reducer=_relu_bias_reducer(b1_sb),
                        mxn_subtile_producer=hT_out_producer,
                        mxn_consumer=_noop_consumer,
                        MAX_TILE_SIZE=CT,
                        psum_n_bufs=2,
                    )

                # prefetch the next expert's xT while this expert computes
                if j + 1 < EPC:
                    _load_xt(j + 1)

                # layer 2: y[C, D] = g * (hT[H,C].T @ W2[H,D])
                tc.swap_default_side()
                with ExitStack() as mctx:
                    w2_pool = mctx.enter_context(
                        tc.tile_pool(
                            name=f"w2p{j}", bufs=k_pool_min_bufs_for_dim(H) + 1
                        )
                    )
                    w2_producer, w2_shape = dma_from_dram_kxn(
                        w2_pool, ins[f"w2_{j}"][:]
                    )
                    composable_matmul_tile_kernel(
                        tc=tc,
                        kxm_shape=ShapeInfo(pdims=((128, H // 128),), fdims=(C,)),
                        kxn_shape=w2_shape,
                        output_type=F32,
                        kxm_producer=hT_kxm_producer,
                        kxn_producer=w2_producer,
                        mxn_subtile_reducer=_gate_scale_reducer(g_sb),
                        mxn_consumer=dma_to_dram_mxn(outs[f"y_{j}"][:]),
                        psum_n_bufs=2,
                    )
    nc.compile()
    return nc, in_names, out_names


def kernel(x, noise, w_gate, w_noise, b_noise, W1, b1, W2, b2):
    global LAST_RESULTS
    x = np.asarray(x, np.float32)
    noise = np.asarray(noise, np.float32)
    w_gate = np.asarray(w_gate, np.float32)
    w_noise = np.asarray(w_noise, np.float32)
    b_noise = np.asarray(b_noise, np.float32)
    W1 = np.asarray(W1, np.float32)
    b1 = np.asarray(b1, np.float32)
    W2 = np.asarray(W2, np.float32)
    b2 = np.asarray(b2, np.float32)

    # ---- host gating + dispatch ----
    top_idx, top_gates = _gating(x, noise, w_gate, w_noise, b_noise)

    counts = np.bincount(top_idx.ravel(), minlength=E)

    # Slot assignment: rank experts by load; the 8 heaviest go to slot 0,
    # the 8 lightest to slot 1, so slot 1's padded capacity is smaller.
    order_desc = np.argsort(-counts, kind="stable")
    slot_of = {}   # expert -> (core, slot)
    expert_at = {}  # (core, slot) -> expert
    for r, e in enumerate(order_desc):
        c, j = (r, 0) if r < NCORES else (r - NCORES, 1)
        slot_of[int(e)] = (c, j)
        expert_at[(c, j)] = int(e)

    fast = bool(np.all(b1 == 0.0))
    bf = ml_dtypes.bfloat16
    W1_bf = W1.astype(bf)  # [E, D, H]
    W2_bf = W2.astype(bf)  # [E, H, D]

    if fast:
        # capacity: exact slot max, padded to a multiple of 8 (>=256)
        def _cap(es):
            return int(np.ceil(max(int(counts[es].max()), 256) / 8) * 8)

        Cs = [_cap(order_desc[:NCORES]), _cap(order_desc[NCORES:])]

        idx_lists = [None] * E
        xts = [None] * E
        for e in range(E):
            C = Cs[slot_of[e][1]]
            rows, which = np.nonzero(top_idx == e)
            idx_lists[e] = rows
            n_e = len(rows)
            g = top_gates[rows, which]
            xt = np.zeros((D, C), bf)
            xt[:, :n_e] = (x[rows] * g[:, None]).astype(bf).T
            xts[e] = xt

        nc, in_names, out_names = _build_program_fast(Cs)

        in_maps = []
        for c in range(NCORES):
            m = {}
            for j in range(EPC):
                e = expert_at[(c, j)]
                m[in_names[f"w1_{j}"]] = W1_bf[e]
                m[in_names[f"w2_{j}"]] = W2_bf[e]
                m[in_names[f"xt_{j}"]] = xts[e]
            in_maps.append(m)

        res = run_bass_kernel_spmd(nc, in_maps, core_ids=list(range(NCORES)))
        LAST_RESULTS = res

        gates_full = np.zeros((B, E), np.float32)
        gates_full[np.arange(B)[:, None], top_idx] = top_gates
        out = gates_full @ b2  # [B, D]
        for e in range(E):
            c, j = slot_of[e]
            yt = np.asarray(res.results[c][out_names[f"yt_{j}"]], np.float32)
            rows = idx_lists[e]
            out[rows] += yt[:, : len(rows)].T
        return out.astype(np.float32)

    # ---- general path (b1 != 0) ----
    def _cap(es):
        return int(np.ceil(max(int(counts[es].max()), 128) / 128) * 128)

    Cs = [_cap(order_desc[:NCORES]), _cap(order_desc[NCORES:])]

    x_bf = x.astype(bf)
    idx_lists = [None] * E
    xts = [None] * E
    gs = [None] * E
    b1s = [None] * E
    for e in range(E):
        C = Cs[slot_of[e][1]]
        rows, which = np.nonzero(top_idx == e)
        idx_lists[e] = rows
        n_e = len(rows)
        xt = np.zeros((D, C), bf)
        xt[:, :n_e] = x_bf[rows].T
        xts[e] = xt
        gpad = np.zeros((C,), np.float32)
        gpad[:n_e] = top_gates[rows, which]
        gs[e] = np.ascontiguousarray(gpad.reshape(C // 128, 128).T)
        b1s[e] = np.ascontiguousarray(b1[e].reshape(H // 128, 128).T)

    nc, in_names, out_names = _build_program_general(Cs)

    in_maps = []
    for c in range(NCORES):
        m = {}
        for j in range(EPC):
            e = expert_at[(c, j)]
            m[in_names[f"w1_{j}"]] = W1_bf[e]
            m[in_names[f"w2_{j}"]] = W2_bf[e]
            m[in_names[f"xt_{j}"]] = xts[e]
            m[in_names[f"b1_{j}"]] = b1s[e]
            m[in_names[f"g_{j}"]] = gs[e]
        in_maps.append(m)

    res = run_bass_kernel_spmd(nc, in_maps, core_ids=list(range(NCORES)))
    LAST_RESULTS = res

    gates_full = np.zeros((B, E), np.float32)
    gates_full[np.arange(B)[:, None], top_idx] = top_gates
    out = gates_full @ b2  # [B, D]
    for e in range(E):
        c, j = slot_of[e]
        y = np.asarray(res.results[c][out_names[f"y_{j}"]], np.float32)
        rows = idx_lists[e]
        out[rows] += y[: len(rows)]
    return out.astype(np.float32)
